# revision 29
# baseline (speedup 1.0000x reference)
"""DeepFilter kernel for Trainium2 (8 NeuronCores, batch-parallel).

Math: the reference shifts input and filter by the SAME (df, dt) tap offset,
so the op factorizes into pointwise products followed by a separable 3x5
zero-padded box sum:
    P_r = ir*fr - ii*fi ; P_i = 2*ir*fi
    out_r = boxsum_3x5(P_r) ; out_i = boxsum_3x5(P_i)
    out = concat([out_r, out_i], axis=1)            # [B, 2F, T]

Per-core layout: F on partitions (3 chunks), T on the free dim (pieces of
TH columns + 2-col halo).  DVE computes the 3 product planes and the pair
sums q(c) = p(c)+p(c+1), all written as bf16 so the pair/sub ops hit DVE's
2x perf mode and the planes take half the SBUF (rel err ~2.5e-3, gate is
2e-2); the 5-tap T-box then needs only 3 shifted matmuls per plane (even
shifts only -- odd moving-operand starts are not supported in the fast PE
modes); TensorE applies the F-box (banded bf16 matmul, sign/scale folded
into the band) accumulating in f32 PSUM; ScalarE copies PSUM->SBUF; HWDGE
DMAs stream HBM, with GPSIMD's SWDGE carrying two of the four loads so
three DMA rings share the traffic.  GPSIMD runs no tensor ops: on HW its
elementwise throughput is far below book rate and it stopped hiding under
the DMA stream.

DMA count is minimized (it dominates on this runtime): whole-row T pieces
(th=4000), r+i planes stored with ONE DMA per piece through a [P, 2, th]
staging tile and a dim-permuted view of out as [b, 2, F, T], and the tail
piece's two batch segments loaded with one DMA per tensor.
"""

import numpy as np

B, F, T = 16, 257, 4000
NCORES = 8
B_LOC = B // NCORES  # 2
P = 128
NT = 500  # psum tile width (<=512 fp32 matmul moving-operand limit)

# Regular F chunks: (first loaded row, n rows loaded,
#                    valid psum partitions [lo,hi), first output f row)
#  c0: rows 0..127   -> f 0..126  at partitions 0..126
#  c1: rows 126..253 -> f 127..252 at partitions 1..126
# The tail (f 253..256) is handled by a merged macro-tile covering BOTH
# batches: partitions b*6+r hold rows 251+r of batch b; a block-diagonal
# [12,8] band produces f 253..256 for b0 at partitions 0..3, b1 at 4..7.
CHUNKS = [
    (0, 128, 0, 127, 0),
    (126, 128, 1, 127, 127),
]
C2_FL0, C2_NROWS_B, C2_FO0, C2_NF = 251, 6, 253, 4


# th=4000 (whole rows, halved DMA count) wins on this platform: the
# effective DMA cost at 8-core saturation has a large per-request term
# (~3-7us/DMA through the tunneled runtime), so fewer, bigger transfers
# beat deeper pipelining (th=2000 R=8 wall 8931us -> th=4000 7477us).
DEFAULT_TH = 4000
DEFAULT_BUFS = dict(inp=6, prod=3, pair=2, stg=2, ps=8)
DEFAULT_PLANE = "bf16"
DEFAULT_OPS = "dve"
DEFAULT_INP = "f32"

_CACHE = {}


def _band_matrices(plane=None):
    plane = DEFAULT_PLANE if plane is None else plane
    if plane == "bf16":
        import ml_dtypes
        dt = ml_dtypes.bfloat16
    else:
        dt = np.float32
    k = np.arange(P)
    band = (np.abs(k[:, None] - k[None, :]) <= 1).astype(dt)
    w6 = np.zeros((12, 8), dt)
    for bb in range(2):
        for r in range(6):
            for m in range(4):
                if abs(m + 2 - r) <= 1:
                    w6[bb * 6 + r, bb * 4 + m] = 1.0
    return band, w6


def _build_module(repeats=1, th=DEFAULT_TH, bufs=None, dma_only=False,
                  plane=None, ops=None, inp=None):
    plane = DEFAULT_PLANE if plane is None else plane
    ops = DEFAULT_OPS if ops is None else ops
    inp = DEFAULT_INP if inp is None else inp
    if dma_only:
        inp = "f32"  # measurement variant forwards input bytes to out
    import concourse.bacc as bacc
    import concourse.mybir as mybir
    import concourse.tile as tile

    bufs = dict(DEFAULT_BUFS, **(bufs or {}))
    assert T % th == 0 and th % NT == 0
    n_pieces = T // th
    nj = th // NT
    tw = th + 4

    f32 = mybir.dt.float32
    f32r = mybir.dt.float32r
    # plane dtype for product/pair tiles + the stationary bands: bf16 runs
    # DVE pair ops in 2x mode and halves plane SBUF, at ~3e-3 rel err
    pdt = mybir.dt.bfloat16 if plane == "bf16" else f32r
    mult = mybir.AluOpType.mult

    nc = bacc.Bacc("TRN2", target_bir_lowering=False, debug=False,
                   num_devices=NCORES)

    ins = {
        name: nc.dram_tensor(name, [B_LOC, F, T], f32, kind="ExternalInput")
        for name in ("inputs_r", "inputs_i", "filters_r", "filters_i")
    }
    wp_d = nc.dram_tensor("wp", [P, P], pdt, kind="ExternalInput")
    w6p_d = nc.dram_tensor("w6p", [12, 8], pdt, kind="ExternalInput")
    out_d = nc.dram_tensor("out", [B_LOC, 2 * F, T], f32, kind="ExternalOutput")

    ir_ap, ii_ap, fr_ap, fi_ap = (ins[n].ap() for n in
                                  ("inputs_r", "inputs_i", "filters_r",
                                   "filters_i"))
    out_ap = out_d.ap()
    # out viewed as [b, plane, F, T] for the merged r+i stores
    out4 = out_d.ap().rearrange("b (h f) t -> b h f t", h=2)

    with tile.TileContext(nc) as tc:
        with (
            tc.tile_pool(name="const", bufs=1) as cpool,
            tc.tile_pool(name="inp", bufs=bufs["inp"]) as ipool,
            tc.tile_pool(name="prod", bufs=bufs["prod"]) as rpool,
            tc.tile_pool(name="pair", bufs=bufs["pair"]) as wpool,
            tc.tile_pool(name="stg", bufs=bufs["stg"]) as spool,
            tc.tile_pool(name="ps", bufs=bufs["ps"], space="PSUM") as qpool,
        ):
            wp_s = cpool.tile([P, P], pdt, name="wp_s", tag="wp_s")
            w6p_s = cpool.tile([12, 8], pdt, name="w6p_s", tag="w6p_s")
            nc.sync.dma_start(out=wp_s[:, :], in_=wp_d.ap()[:, :])
            nc.sync.dma_start(out=w6p_s[:, :], in_=w6p_d.ap()[:, :])

            def emit_piece(h, loads, nrows, wpL, np_out, vp1, stores):
                """One macro-tile: T piece h, given per-batch loads
                [(part_off, b, fl0, nr)], band slices, valid psum rows
                [0,vp1), and stores [(stage p0, p1, b, first f row)]."""
                t0 = th * h
                # tile col c <-> t = t0 - 2 + c ; clip to [0, T)
                c_lo = max(0, 2 - t0)
                c_hi = tw - max(0, t0 + th + 2 - T)
                t_lo, t_hi = t0 - 2 + c_lo, t0 - 2 + c_hi

                idt = f32 if inp == "f32" else mybir.dt.bfloat16
                ir_t = ipool.tile([P, tw], idt, name="ir_t", tag="inp")
                ii_t = ipool.tile([P, tw], idt, name="ii_t", tag="inp")
                fr_t = ipool.tile([P, tw], idt, name="fr_t", tag="inp")
                fi_t = ipool.tile([P, tw], idt, name="fi_t", tag="inp")
                # inp=="f32": loads split across two DMA issue paths (SP
                # HWDGE + GPSIMD SWDGE) so transfers from different tensors
                # can overlap instead of sitting FIFO in one ring.
                # inp=="bf16": the f32->bf16 cast happens inside the DMA,
                # which only SWDGE supports -- all four loads go out the
                # GPSIMD ring (HBM-side traffic is unchanged; the SBUF
                # write side halves), and the stores move to the two HWDGE
                # rings to keep the ring split balanced.
                if inp == "f32":
                    load_plan = ((ir_t, ir_ap, nc.sync),
                                 (ii_t, ii_ap, nc.gpsimd),
                                 (fr_t, fr_ap, nc.sync),
                                 (fi_t, fi_ap, nc.gpsimd))
                else:
                    load_plan = ((ir_t, ir_ap, nc.gpsimd),
                                 (ii_t, ii_ap, nc.gpsimd),
                                 (fr_t, fr_ap, nc.gpsimd),
                                 (fi_t, fi_ap, nc.gpsimd))
                for t_sb, src, eng in load_plan:
                    if len(loads) == 2 and loads[0][2] == loads[1][2]:
                        # merged tail: both batches' row segments in ONE
                        # DMA -- src (b, row, col) flattens straight onto
                        # partitions p = b*nr + row (batch-outer order)
                        _, _, fl0, nr = loads[0]
                        eng.dma_start(
                            out=t_sb[0:2 * nr, c_lo:c_hi],
                            in_=src[0:B_LOC, fl0:fl0 + nr, t_lo:t_hi])
                    else:
                        for p_off, b, fl0, nr in loads:
                            eng.dma_start(
                                out=t_sb[p_off:p_off + nr, c_lo:c_hi],
                                in_=src[b, fl0:fl0 + nr, t_lo:t_hi])
                    # zero halo cols at the global T edges so the products
                    # are zero there (zero-pad semantics) and matmuls can
                    # always run full-width (fp32r needs even widths)
                    if c_lo > 0:
                        nc.vector.memset(t_sb[0:nrows, 0:c_lo], 0.0)
                    if c_hi < tw:
                        nc.vector.memset(t_sb[0:nrows, c_hi:tw], 0.0)

                if dma_only:
                    # measurement variant: identical DMA traffic, no
                    # compute -- stores forward slices of the loads
                    for sp0, sp1, b, fo0 in stores:
                        n_f = sp1 - sp0
                        nc.scalar.dma_start(
                            out=out_ap[b, fo0:fo0 + n_f, t0:t0 + th],
                            in_=ir_t[sp0:sp1, 2:2 + th])
                        nc.scalar.dma_start(
                            out=out_ap[b, F + fo0:F + fo0 + n_f, t0:t0 + th],
                            in_=ii_t[sp0:sp1, 2:2 + th])
                    return

                # Product planes in `pdt`: PE runs its 1-col/cycle mode for
                # both f32r and bf16; with bf16 the pair/sub ops also hit
                # DVE's 2x perf mode.  Combining pr = t1 - t2 on the vector
                # engines (instead of a negative band on PE) means both
                # planes share the ONE wp band: 6 matmuls per psum pair,
                # zero LDW switches.
                t1_t = rpool.tile([P, tw], pdt, name="t1_t", tag="prod")
                t2_t = rpool.tile([P, tw], pdt, name="t2_t", tag="prod")
                pi_t = rpool.tile([P, tw], pdt, name="pi_t", tag="prod")
                # ops="dve": everything on DVE, GPSIMD keeps only SWDGE
                # descgen (HW GPSIMD tensor ops run well below the book
                # rate and can stop hiding under the DMA stream).
                sub_eng = nc.vector if ops == "dve" else nc.gpsimd
                qr_eng = nc.vector if ops == "dve" else nc.gpsimd
                nc.vector.tensor_mul(t1_t[0:nrows, 0:tw],
                                     ir_t[0:nrows, 0:tw],
                                     fr_t[0:nrows, 0:tw])
                nc.vector.tensor_mul(t2_t[0:nrows, 0:tw],
                                     ii_t[0:nrows, 0:tw],
                                     fi_t[0:nrows, 0:tw])
                # pi = (ir * 2) * fi -- fold the reference's factor 2
                nc.vector.scalar_tensor_tensor(
                    out=pi_t[0:nrows, 0:tw],
                    in0=ir_t[0:nrows, 0:tw], scalar=2.0,
                    in1=fi_t[0:nrows, 0:tw], op0=mult, op1=mult)
                # pr = t1 - t2, in place over t1
                pr_t = t1_t
                sub_eng.tensor_sub(pr_t[0:nrows, 0:tw],
                                   t1_t[0:nrows, 0:tw],
                                   t2_t[0:nrows, 0:tw])

                # pair sums q(c) = p(c) + p(c+1): the 5-tap T-box then
                # needs only 3 matmuls per plane (q(t-2) + q(t) + p(t+2)).
                qr_t = wpool.tile([P, tw], pdt, name="qr_t", tag="pair")
                qi_t = wpool.tile([P, tw], pdt, name="qi_t", tag="pair")
                qr_eng.tensor_add(qr_t[0:nrows, 0:tw - 1],
                                  pr_t[0:nrows, 0:tw - 1],
                                  pr_t[0:nrows, 1:tw])
                nc.vector.tensor_add(qi_t[0:nrows, 0:tw - 1],
                                     pi_t[0:nrows, 0:tw - 1],
                                     pi_t[0:nrows, 1:tw])
                # second level, in place: r(c) = q(c) + q(c+2)
                #   = p(c)+p(c+1)+p(c+2)+p(c+3), so the 5-tap T-box is
                # just r(t-2) + p(t+2): 2 matmuls per plane per j.  The
                # write cursor trails the +2 read offset, so the in-place
                # streaming update is hazard-free and costs no extra SBUF.
                qr_eng.tensor_add(qr_t[0:nrows, 0:tw - 3],
                                  qr_t[0:nrows, 0:tw - 3],
                                  qr_t[0:nrows, 2:tw - 1])
                nc.vector.tensor_add(qi_t[0:nrows, 0:tw - 3],
                                     qi_t[0:nrows, 0:tw - 3],
                                     qi_t[0:nrows, 2:tw - 1])
                # third level, in place again: s(c) = r(c) + p(c+4) is
                # the full 5-tap box, so out(t) = s(t-2): ONE matmul per
                # plane per psum tile.
                qr_eng.tensor_add(qr_t[0:nrows, 0:tw - 5],
                                  qr_t[0:nrows, 0:tw - 5],
                                  pr_t[0:nrows, 4:tw - 1])
                nc.vector.tensor_add(qi_t[0:nrows, 0:tw - 5],
                                     qi_t[0:nrows, 0:tw - 5],
                                     pi_t[0:nrows, 4:tw - 1])

                # one [P, 2, th] staging tile: plane index on the middle
                # dim so ONE store DMA per piece covers both r and i
                # (the out tensor viewed as [b, 2, F, T] via rearrange)
                stg_t = spool.tile([P, 2, th], f32, name="stg_t", tag="stg")

                for j in range(nj):
                    ps_r = qpool.tile([P, NT], f32, name="ps_r", tag="ps")
                    ps_i = qpool.tile([P, NT], f32, name="ps_i", tag="ps")
                    # out(t) = s(t-2) per plane; one shared wp band ->
                    # no LDW switches at all
                    groups = (
                        (ps_i, ((qi_t, wpL, NT * j),)),
                        (ps_r, ((qr_t, wpL, NT * j),)),
                    )
                    for ps, mms in groups:
                        for k, (plane, wL, c_start) in enumerate(mms):
                            nc.tensor.matmul(
                                ps[0:np_out, 0:NT],
                                wL,
                                plane[0:nrows, c_start:c_start + NT],
                                start=(k == 0),
                                stop=(k == len(mms) - 1))
                    # PSUM reads must start at partition 0: copy rows
                    # 0:vp1 and let the store DMAs pick their slices.
                    nc.scalar.copy(
                        out=stg_t[0:vp1, 0, NT * j:NT * (j + 1)],
                        in_=ps_r[0:vp1, 0:NT])
                    nc.scalar.copy(
                        out=stg_t[0:vp1, 1, NT * j:NT * (j + 1)],
                        in_=ps_i[0:vp1, 0:NT])

                # stores go out the ACT HWDGE ring so they queue behind
                # their producing copies instead of blocking the SP ring's
                # input loads (head-of-line).  One DMA per piece: the out
                # tensor viewed as [b, 2, F, T]; dim permutations make the
                # HBM-side flatten order match stg_t's (part, plane, col).
                for sp0, sp1, b, fo0 in stores:
                    n_f = sp1 - sp0
                    nc.scalar.dma_start(
                        out=out4[b, 0:2, fo0:fo0 + n_f, t0:t0 + th]
                        .transpose([1, 0, 2]),
                        in_=stg_t[sp0:sp1, 0:2, 0:th])

            # piece sequence: regular (b, chunk, h) pieces, with the
            # DMA-light merged-tail pieces interleaved mid-stream
            for _rep in range(repeats):
                half = (n_pieces + 1) // 2
                for b in range(B_LOC):
                    for fl0, nrows, vp0, vp1, fo0 in CHUNKS:
                        for h in range(n_pieces):
                            emit_piece(
                                h, [(0, b, fl0, nrows)], nrows,
                                wp_s[:, :], P, vp1,
                                [(vp0, vp1, b, fo0)])
                    # merged tail pieces: first half after batch 0,
                    # second half after batch 1
                    hs = range(0, half) if b == 0 else range(half, n_pieces)
                    for h in hs:
                        emit_piece(
                            h,
                            [(0, 0, C2_FL0, C2_NROWS_B),
                             (6, 1, C2_FL0, C2_NROWS_B)],
                            12, w6p_s[0:12, 0:8], 8, 8,
                            [(0, 4, 0, C2_FO0), (4, 8, 1, C2_FO0)])

    nc.compile()
    return nc


def _get_module(repeats=1, th=DEFAULT_TH, bufs=None, dma_only=False,
                plane=None, ops=None, inp=None):
    key = (f"nc{repeats}_{th}_{sorted((bufs or {}).items())}_{dma_only}"
           f"_{plane or DEFAULT_PLANE}_{ops or DEFAULT_OPS}"
           f"_{inp or DEFAULT_INP}")
    if key not in _CACHE:
        _CACHE[key] = _build_module(repeats, th, bufs, dma_only, plane, ops,
                                    inp)
    return _CACHE[key]


def _runner():
    """Build (once) a reusable jitted 8-core runner for the module."""
    if "runner" in _CACHE:
        return _CACHE["runner"]
    import jax
    import concourse.mybir as mybir
    from concourse import bass2jax
    from jax.sharding import Mesh, NamedSharding, PartitionSpec
    from jax.experimental.shard_map import shard_map

    nc = _get_module()
    bass2jax.install_neuronx_cc_hook()

    partition_name = (nc.partition_id_tensor.name
                      if nc.partition_id_tensor else None)
    in_names, out_names, out_avals, zero_outs = [], [], [], []
    for alloc in nc.m.functions[0].allocations:
        if not isinstance(alloc, mybir.MemoryLocationSet):
            continue
        name = alloc.memorylocations[0].name
        if alloc.kind == "ExternalInput":
            if name != partition_name:
                in_names.append(name)
        elif alloc.kind == "ExternalOutput":
            out_names.append(name)
            shape = tuple(alloc.tensor_shape)
            dtype = mybir.dt.np(alloc.dtype)
            out_avals.append(jax.core.ShapedArray(shape, dtype))
            zero_outs.append(np.zeros(shape, dtype))
    n_params = len(in_names)
    all_in_names = list(in_names) + list(out_names)
    if partition_name is not None:
        all_in_names.append(partition_name)

    def _body(*args):
        operands = list(args)
        if partition_name is not None:
            operands.append(bass2jax.partition_id_tensor())
        return tuple(bass2jax._bass_exec_p.bind(
            *operands,
            out_avals=tuple(out_avals),
            in_names=tuple(all_in_names),
            out_names=tuple(out_names),
            lowering_input_output_aliases=(),
            sim_require_finite=True,
            sim_require_nnan=True,
            nc=nc,
        ))

    devices = jax.devices()[:NCORES]
    mesh = Mesh(np.asarray(devices), ("core",))
    n_outs = len(out_names)
    in_specs = (PartitionSpec("core"),) * (n_params + n_outs)
    out_specs = (PartitionSpec("core"),) * n_outs
    f = jax.jit(shard_map(_body, mesh=mesh, in_specs=in_specs,
                          out_specs=out_specs, check_rep=False),
                keep_unused=True)
    sharding = NamedSharding(mesh, PartitionSpec("core"))
    dev_zero = [
        jax.device_put(np.concatenate([z] * NCORES, axis=0), sharding)
        for z in zero_outs
    ]
    _CACHE["runner"] = (f, sharding, in_names, out_names, dev_zero)
    return _CACHE["runner"]


def kernel(**inputs):
    import jax

    f, sharding, in_names, out_names, dev_zero = _runner()
    wp, w6p = _band_matrices()
    consts = {"wp": np.concatenate([wp] * NCORES, axis=0),
              "w6p": np.concatenate([w6p] * NCORES, axis=0)}
    dev_in = []
    for nm in in_names:
        arr = consts[nm] if nm in consts else np.ascontiguousarray(inputs[nm])
        dev_in.append(jax.device_put(arr, sharding))
    outs = f(*dev_in, *dev_zero)
    out = np.asarray(outs[out_names.index("out")])
    return out

